# revision 20
# baseline (speedup 1.0000x reference)
"""MoE (8 experts, top-2) + shared-expert SwiGLU on 8 TRN2 NeuronCores.

Expert-parallel: core e holds expert e's weights; host routes tokens
(bit-exact jax-CPU replica of the reference gate) and gathers each
expert's tokens. Shared expert is token-sharded (512 tokens/core).
Device math is bf16 with fp32 PSUM accumulation, transposed layout
(no on-device transposes). w3 stays SBUF-resident per phase; w1/w2
stream per hidden chunk. Per-core token counts are runtime For_i trip
counts (full 512-token blocks + 128-token remainder blocks), so each
core computes only ~its actual load instead of a worst-case cap.
Host applies top-k gate weights and scatters expert outputs back.
"""

import time
from contextlib import ExitStack

import numpy as np
import ml_dtypes

import concourse.tile as tile
from concourse import bacc, mybir
from concourse.bass import ds

BF16 = ml_dtypes.bfloat16
P = 128
S = 4096
C = 1024
E = 8
TOP_K = 2
H = 2744
HP = 2816          # H padded to 22*128
NH = HP // P       # 22 hidden chunks
NCC = C // P       # 8 channel chunks
TB = 512           # tokens per full block
RB = 128           # tokens per remainder block
RW = 384           # remainder region width (up to 3 RB blocks)
FW_DEF = 1536      # default full region width (3 TB blocks)
N_CORES = 8

ACT_FN = "Silu"
LAST_EXEC_NS = None
LAST_TIMES = []
LAST_META = None
_REPEAT = 1
_CACHE = {}


def _routing(xf32, gate_w):
    """Replicate reference gate math bit-exactly on jax-CPU."""
    import jax
    import jax.numpy as jnp

    cpu = jax.devices("cpu")[0]
    with jax.default_device(cpu):
        xj = jnp.asarray(xf32)
        logits = xj @ jnp.asarray(gate_w).T
        rw = jax.nn.softmax(logits.astype(jnp.float32), axis=-1)
        tkw, tki = jax.lax.top_k(rw, TOP_K)
        importance = rw.mean(0)
        load = jax.nn.one_hot(tki[:, 0], E, dtype=jnp.float32).mean(0)
        aux = E * (importance * load).sum()
        tkw = tkw / tkw.sum(-1, keepdims=True)
        tki_np = np.asarray(tki)
        tkw_np = np.asarray(tkw, dtype=np.float32)
        aux_np = np.float32(aux)
    return tki_np, tkw_np, aux_np


_UID = [0]


def _emit_block(nc, tc, pools, W, src, i, wS1, wS2, wc, dst):
    """dst[:, i:i+W] = swiglu(src[:, i:i+W]); i may be a loop var."""
    AF = mybir.ActivationFunctionType
    bf = mybir.dt.bfloat16
    f32 = mybir.dt.float32
    wsp, xp, hp, silp, stp, psh, pse = pools
    _UID[0] += 1
    u = _UID[0]
    if isinstance(i, int):
        def sl(t, c):
            return t[c * P:(c + 1) * P, i:i + W]
    else:
        def sl(t, c):
            return t[c * P:(c + 1) * P, ds(i, W)]

    xt = xp.tile([P, NCC * W], bf, name=f"xt{u}", tag="xt")
    for c in range(NCC):
        nc.sync.dma_start(xt[:, c * W:(c + 1) * W], sl(src, c))

    ht = hp.tile([P, NH * W], bf, name=f"ht{u}", tag="ht")
    for hc in range(NH):
        w1t = wsp.tile([P, NCC * P], bf, name=f"w1t{u}_{hc}", tag="w1t")
        w2t = wsp.tile([P, NCC * P], bf, name=f"w2t{u}_{hc}", tag="w2t")
        nc.sync.dma_start(w1t[:], wS1[hc * P:(hc + 1) * P, :])
        nc.sync.dma_start(w2t[:], wS2[hc * P:(hc + 1) * P, :])
        ps1 = psh.tile([P, W], f32, name=f"ps1{u}_{hc}", tag="ps1")
        ps2 = psh.tile([P, W], f32, name=f"ps2{u}_{hc}", tag="ps2")
        for c in range(NCC):
            nc.tensor.matmul(
                ps1[:], lhsT=w1t[:, c * P:(c + 1) * P],
                rhs=xt[:, c * W:(c + 1) * W],
                start=(c == 0), stop=(c == NCC - 1),
            )
        for c in range(NCC):
            nc.tensor.matmul(
                ps2[:], lhsT=w2t[:, c * P:(c + 1) * P],
                rhs=xt[:, c * W:(c + 1) * W],
                start=(c == 0), stop=(c == NCC - 1),
            )
        sil = silp.tile([P, W], bf, name=f"sil{u}_{hc}", tag="sil")
        nc.scalar.activation(sil[:], ps1[:], getattr(AF, ACT_FN))
        nc.vector.tensor_tensor(
            out=ht[:, hc * W:(hc + 1) * W],
            in0=sil[:], in1=ps2[:], op=mybir.AluOpType.mult,
        )

    for c in range(NCC):
        eop = pse.tile([P, W], f32, name=f"eop{u}_{c}", tag="eop")
        for hc in range(NH):
            nc.tensor.matmul(
                eop[:],
                lhsT=wc[:, hc * C + c * P: hc * C + c * P + P],
                rhs=ht[:, hc * W:(hc + 1) * W],
                start=(hc == 0), stop=(hc == NH - 1),
            )
        st = stp.tile([P, W], f32, name=f"st{u}_{c}", tag="st")
        nc.scalar.activation(st[:], eop[:], AF.Copy)
        nc.sync.dma_start(sl(dst, c), st[:])


def _build_program(fw, repeat=1, static_counts=None):
    nc = bacc.Bacc("TRN2", target_bir_lowering=False, debug=False,
                   num_devices=N_CORES)
    bf = mybir.dt.bfloat16
    f32 = mybir.dt.float32
    d_xgF = nc.dram_tensor("xgF", [C, fw], bf, kind="ExternalInput")
    d_xgR = nc.dram_tensor("xgR", [C, RW], bf, kind="ExternalInput")
    d_xsT = nc.dram_tensor("xsT", [C, TB], bf, kind="ExternalInput")
    d_w1S = nc.dram_tensor("w1S", [NH * P, NCC * P], bf, kind="ExternalInput")
    d_w2S = nc.dram_tensor("w2S", [NH * P, NCC * P], bf, kind="ExternalInput")
    d_w3T = nc.dram_tensor("w3T", [HP, C], bf, kind="ExternalInput")
    d_s1S = nc.dram_tensor("s1S", [NH * P, NCC * P], bf, kind="ExternalInput")
    d_s2S = nc.dram_tensor("s2S", [NH * P, NCC * P], bf, kind="ExternalInput")
    d_s3T = nc.dram_tensor("s3T", [HP, C], bf, kind="ExternalInput")
    d_meta = nc.dram_tensor("meta", [1, 2], mybir.dt.int32, kind="ExternalInput")
    d_eoF = nc.dram_tensor("eoF", [C, fw], f32, kind="ExternalOutput")
    d_eoR = nc.dram_tensor("eoR", [C, RW], f32, kind="ExternalOutput")
    d_ysT = nc.dram_tensor("ysT", [C, TB], f32, kind="ExternalOutput")

    with tile.TileContext(nc) as tc, ExitStack() as ctx:
        mp = ctx.enter_context(tc.tile_pool(name="mp", bufs=1))
        wcp = ctx.enter_context(tc.tile_pool(name="wcp", bufs=1))
        pools = (
            ctx.enter_context(tc.tile_pool(name="wsp", bufs=3)),
            ctx.enter_context(tc.tile_pool(name="xp", bufs=1)),
            ctx.enter_context(tc.tile_pool(name="hp", bufs=1)),
            ctx.enter_context(tc.tile_pool(name="silp", bufs=2)),
            ctx.enter_context(tc.tile_pool(name="stp", bufs=4)),
            ctx.enter_context(tc.tile_pool(name="psh", bufs=2, space="PSUM")),
            ctx.enter_context(tc.tile_pool(name="pse", bufs=3, space="PSUM")),
        )

        if static_counts is None:
            mt = mp.tile([1, 2], mybir.dt.int32, name="mt")
            nc.sync.dma_start(mt[:], d_meta[:])
            nf_tok = nc.values_load(mt[0:1, 0:1], min_val=0, max_val=fw,
                                    skip_runtime_bounds_check=True)
            nr_tok = nc.values_load(mt[0:1, 1:2], min_val=0, max_val=RW,
                                    skip_runtime_bounds_check=True)
        else:
            nf_tok, nr_tok = static_counts

        wce = wcp.tile([P, NH * C], bf, name="wce", tag="wce")
        wcs = wcp.tile([P, NH * C], bf, name="wcs", tag="wcs")
        for hc in range(NH):
            nc.sync.dma_start(wce[:, hc * C:(hc + 1) * C],
                              d_w3T[hc * P:(hc + 1) * P, :])
            nc.sync.dma_start(wcs[:, hc * C:(hc + 1) * C],
                              d_s3T[hc * P:(hc + 1) * P, :])

        def body():
            if static_counts is None:
                with tc.For_i(0, nf_tok, TB) as i:
                    _emit_block(nc, tc, pools, TB, d_xgF, i,
                                d_w1S, d_w2S, wce, d_eoF)
                with tc.For_i(0, nr_tok, RB) as i:
                    _emit_block(nc, tc, pools, RB, d_xgR, i,
                                d_w1S, d_w2S, wce, d_eoR)
            else:
                for i in range(0, nf_tok, TB):
                    _emit_block(nc, tc, pools, TB, d_xgF, i,
                                d_w1S, d_w2S, wce, d_eoF)
                for i in range(0, nr_tok, RB):
                    _emit_block(nc, tc, pools, RB, d_xgR, i,
                                d_w1S, d_w2S, wce, d_eoR)
            _emit_block(nc, tc, pools, TB, d_xsT, 0, d_s1S, d_s2S, wcs, d_ysT)

        if repeat > 1:
            with tc.For_i(0, repeat):
                body()
        else:
            body()

    nc.finalize()
    return nc


def _make_runner(nc, n_cores):
    import jax
    from concourse import bass2jax
    from jax.experimental.shard_map import shard_map
    from jax.sharding import Mesh, PartitionSpec

    bass2jax.install_neuronx_cc_hook()
    partition_name = nc.partition_id_tensor.name if nc.partition_id_tensor else None

    in_names, out_names, out_avals, zero_shapes = [], [], [], []
    for alloc in nc.m.functions[0].allocations:
        if not isinstance(alloc, mybir.MemoryLocationSet):
            continue
        name = alloc.memorylocations[0].name
        if alloc.kind == "ExternalInput":
            if name != partition_name:
                in_names.append(name)
        elif alloc.kind == "ExternalOutput":
            out_names.append(name)
            shape = tuple(alloc.tensor_shape)
            dtype = mybir.dt.np(alloc.dtype)
            out_avals.append(jax.core.ShapedArray(shape, dtype))
            zero_shapes.append((shape, dtype))
    n_params = len(in_names)
    n_outs = len(out_names)
    all_names = list(in_names) + list(out_names)
    if partition_name is not None:
        all_names.append(partition_name)
    all_names = tuple(all_names)
    donate = tuple(range(n_params, n_params + n_outs))

    def _body(*args):
        operands = list(args)
        if partition_name is not None:
            operands.append(bass2jax.partition_id_tensor())
        outs = bass2jax._bass_exec_p.bind(
            *operands,
            out_avals=tuple(out_avals),
            in_names=all_names,
            out_names=tuple(out_names),
            lowering_input_output_aliases=(),
            sim_require_finite=True,
            sim_require_nnan=True,
            nc=nc,
        )
        return tuple(outs)

    devices = jax.devices()[:n_cores]
    mesh = Mesh(np.asarray(devices), ("core",))
    in_specs = (PartitionSpec("core"),) * (n_params + n_outs)
    out_specs = (PartitionSpec("core"),) * n_outs
    sharded = jax.jit(
        shard_map(_body, mesh=mesh, in_specs=in_specs,
                  out_specs=out_specs, check_rep=False),
        donate_argnums=donate, keep_unused=True,
    )
    return sharded, in_names, out_names, out_avals, zero_shapes, mesh


def _run(nc, in_maps, timing_reps=3):
    global LAST_EXEC_NS, LAST_TIMES
    import jax
    from jax.sharding import NamedSharding, PartitionSpec

    key = ("runner", id(nc))
    if key not in _CACHE:
        _CACHE[key] = _make_runner(nc, N_CORES)
    sharded, in_names, out_names, out_avals, zero_shapes, mesh = _CACHE[key]

    sh = NamedSharding(mesh, PartitionSpec("core"))
    concat_in = [
        np.concatenate([np.asarray(in_maps[c][nm]) for c in range(N_CORES)], axis=0)
        for nm in in_names
    ]
    dev_in = [jax.device_put(a, sh) for a in concat_in]

    def zero_set():
        return [
            jax.device_put(np.zeros((N_CORES * s[0], *s[1:]), dt), sh)
            for s, dt in zero_shapes
        ]

    zsets = [zero_set() for _ in range(timing_reps + 1)]
    jax.block_until_ready((dev_in, zsets))

    out_arrs = sharded(*dev_in, *zsets[0])
    jax.block_until_ready(out_arrs)
    results = [
        {name: np.asarray(out_arrs[i]).reshape(N_CORES, *out_avals[i].shape)[c]
         for i, name in enumerate(out_names)}
        for c in range(N_CORES)
    ]

    times = []
    for zs in zsets[1:]:
        t0 = time.perf_counter()
        o = sharded(*dev_in, *zs)
        jax.block_until_ready(o)
        times.append(time.perf_counter() - t0)
    LAST_TIMES = times
    if times:
        LAST_EXEC_NS = int(min(times) * 1e9)
    return results


def _stream_layout(w):
    """[H, C] row-major weight -> [NH*P, NCC*P] per-hc lhsT tiles."""
    wp = np.zeros((HP, C), BF16)
    wp[:H] = np.asarray(w, np.float32).astype(BF16)
    t = wp.reshape(NH, P, NCC, P).transpose(0, 3, 2, 1)
    return np.ascontiguousarray(t.reshape(NH * P, NCC * P))


def _wcT(w):
    """[C, H] weight -> transposed padded [HP, C]."""
    out = np.zeros((HP, C), BF16)
    out[:H] = np.asarray(w, np.float32).astype(BF16).T
    return out


def kernel(x, gate_w, w1, w2, w3, sw1, sw2, sw3):
    global LAST_META
    x = np.asarray(x, dtype=np.float32)
    gate_w = np.asarray(gate_w, dtype=np.float32)
    xf32 = np.ascontiguousarray(x.reshape(S, C))

    tki, tkw, aux = _routing(xf32, gate_w)

    sel = np.zeros((S, E), dtype=bool)
    for k in range(TOP_K):
        sel[np.arange(S), tki[:, k]] = True
    counts = sel.sum(0)
    toks = [np.nonzero(sel[:, e])[0] for e in range(E)]

    plans = []
    for e in range(E):
        cnt = int(counts[e])
        nf = cnt // TB
        rem = cnt - nf * TB
        if rem > RW:
            nf += 1
            rem = 0
        nthin_tok = -(-rem // RB) * RB
        plans.append((cnt, nf, rem, nthin_tok))
    fw = max(FW_DEF, max(p[1] for p in plans) * TB)
    LAST_META = plans

    x_bf = xf32.astype(BF16)
    s1S = _stream_layout(sw1)
    s2S = _stream_layout(sw2)
    s3T = _wcT(sw3)

    in_maps = []
    for e in range(E):
        cnt, nf, rem, nthin_tok = plans[e]
        full_used = min(cnt, nf * TB)
        xgF = np.zeros((C, fw), BF16)
        xgF[:, :full_used] = x_bf[toks[e][:full_used]].T
        xgR = np.zeros((C, RW), BF16)
        if rem:
            xgR[:, :rem] = x_bf[toks[e][full_used:]].T
        xsT = np.ascontiguousarray(x_bf[TB * e:TB * (e + 1)].T)
        meta = np.array([[nf * TB, nthin_tok]], dtype=np.int32)
        in_maps.append({
            "xgF": xgF, "xgR": xgR, "xsT": xsT,
            "w1S": _stream_layout(w1[e]), "w2S": _stream_layout(w2[e]),
            "w3T": _wcT(w3[e]),
            "s1S": s1S, "s2S": s2S, "s3T": s3T,
            "meta": meta,
        })

    key = ("nc", fw, _REPEAT)
    if key not in _CACHE:
        _CACHE[key] = _build_program(fw, repeat=_REPEAT)
    nc = _CACHE[key]

    results = _run(nc, in_maps)

    y = np.empty((S, C), dtype=np.float32)
    for e in range(E):
        y[TB * e:TB * (e + 1)] = results[e]["ysT"].T
    for e in range(E):
        t = toks[e]
        if len(t) == 0:
            continue
        cnt, nf, rem, _ = plans[e]
        full_used = min(cnt, nf * TB)
        eo = np.empty((cnt, C), np.float32)
        eo[:full_used] = results[e]["eoF"][:, :full_used].T
        if rem:
            eo[full_used:] = results[e]["eoR"][:, :rem].T
        ke = np.where(tki[t, 0] == e, 0, 1)
        wv = tkw[t, ke].astype(np.float32)
        y[t] += eo * wv[:, None]

    return y.reshape(4, 1024, C), aux
